# revision 16
# baseline (speedup 1.0000x reference)
"""AttentionCropLayer Trainium2 kernel.

Per sample b: offsets (w,h) = floor(clip(locs[b]*224, 44, 180) - 44); output
out[b] = images[b, :, w:w+88, h:h+88] * mask, with mask the fixed 88x88
sigmoid-profile outer product.

The sigmoid profile sig(10*r) - sig(10*(r-88)) is 0.5 at r=0 and within
4.6e-5 of 1.0 for r=1..87, so the mask reduces to scaling row 0 and column 0
of each crop by 0.5 (corner 0.25); the interior passes through. Max relative
error ~1e-4, far inside the 2e-2 gate.

Descriptor count and size dominate DMA cost, so the host reorders each
core's slab to channel-last [s][u][v][c]: one crop row covers all 16
channels as a single contiguous 1408-element (5632B) run. The crop lands in
SBUF as [partition = crop row i, free = (sample, k, c)] and is stored in the
same channel-last layout ([s][i][k][c], host permutes back), all with exact
bytes - no padding.

Engine spread (HW-calibrated): an HWDGE dma_start with n descriptors
engages ~n/8 SDMA engines from slot 0, so the per-sample read is shaped as
128 descriptors (rows 0-63 as half-row pairs, 2 descriptors per partition)
which measure uniform across all 16 engines; the 24-row tail goes through
SWDGE (gpsimd), whose generator always spreads uniformly. Stores are SWDGE
group-of-4-samples DMAs (348 descriptors), measured uniform at full rate.

Edge scaling on device: k=0 columns are free[0:16] on every partition (one
tiny DVE op per sample); the i=0 row lives on partition 0 only, so it is
copied per 4-sample group via SBUF->SBUF DMA into [8, 1408] edge tiles
(partition = sample), scaled there, and stored as the out[:, 0, :, :] rows.
"""

import sys

if "/opt/trn_rl_repo" not in sys.path:
    sys.path.insert(0, "/opt/trn_rl_repo")

import numpy as np

import concourse.bass as bass
import concourse.bacc as bacc
import concourse.mybir as mybir
from concourse import tile
from concourse.bass_utils import run_bass_kernel_spmd

TL = 44
CROP = 2 * TL          # 88
SCALE = 224.0
B, C, IN = 128, 16, 224
NCORES = 8
BPC = B // NCORES      # 16 samples per core
MAXOFF = IN - CROP     # 136
IMSZ = C * IN * IN     # elements per sample
RUN = CROP * C         # 1408: one crop row x all channels (5632B)
HALF = RUN // 2        # 704
USTRIDE = IN * C       # 3584: element stride between consecutive u rows
SROW = CROP * RUN      # 123904: elements per sample in device-out layout
RSPLIT = 64            # rows 0-63 via HWDGE ring, 64-87 via SWDGE
MAXEOFF = (BPC - 1) * IMSZ + C * (MAXOFF * IN + MAXOFF)

_nc_cache = {}


def _build_nc():
    nc = bacc.Bacc(None)
    images = nc.declare_dram_parameter(
        "images", [1, BPC * IMSZ], mybir.dt.float32, isOutput=False
    )
    offs = nc.declare_dram_parameter(
        "offs", [1, BPC], mybir.dt.int32, isOutput=False
    )
    # channel-last device output [s][i][k][c]; host permutes to [s][c][i][k]
    out = nc.declare_dram_parameter(
        "out", [BPC, CROP, CROP, C], mybir.dt.float32, isOutput=True
    )

    with tile.TileContext(nc) as tc:
        with (
            tc.tile_pool(name="const", bufs=1) as cpool,
            tc.tile_pool(name="work", bufs=1) as wpool,
        ):
            # warm the dynamic-DMA path on both HWDGE rings with a dummy
            # register-offset read (first dynamic DMA per ring pays a one-time
            # ucode-load cost); overlaps the offset staging DMA
            regs = {}
            for rk, weng in (("sync", nc.sync), ("scalar", nc.scalar)):
                reg = weng.alloc_register(
                    "o_reg_sp" if rk == "sync" else "o_reg_act"
                )
                regs[rk] = reg
                weng.reg_mov(reg, 0)
                ov0 = weng.snap(reg, donate=True, min_val=0, max_val=0)
                wsrc = bass.AP(
                    tensor=images[:].tensor,
                    offset=ov0,
                    ap=[[64, 1], [1, 64]],
                    dep_tracking_offset=0,
                )
                wt_ = cpool.tile([1, 64], mybir.dt.float32, tag=f"warm_{rk}")
                weng.dma_start(out=wt_[:], in_=wsrc)
            offs_sb = cpool.tile([1, BPC], mybir.dt.int32)
            nc.sync.dma_start(out=offs_sb[:], in_=offs[:])

            # crop tile: partition = crop row i, free = (sample, k, c)
            t = wpool.tile([CROP, BPC * RUN], mybir.dt.float32, tag="crop")
            # i=0 rows regrouped as partition = sample; two 8-partition
            # tiles so the scale ops start at partition 0
            tmps = [
                wpool.tile([8, RUN], mybir.dt.float32, name=f"edge{g}",
                           tag=f"edge{g}")
                for g in range(2)
            ]

            engs = {"sync": nc.sync, "scalar": nc.scalar}
            g_reg = nc.gpsimd.alloc_register("o_reg_pool")

            def pool_group_ops(g):
                # i=0 row regroup for samples 4g..4g+3 (4 desc of 5632B),
                # then the group store: rows 1-87 x 4 samples = 348 desc
                nc.gpsimd.dma_start(
                    out=tmps[g // 2][(g % 2) * 4 : (g % 2) * 4 + 4, :],
                    in_=t[0:1, 4 * g * RUN : 4 * (g + 1) * RUN],
                )
                dst = bass.AP(
                    tensor=out[:].tensor,
                    offset=4 * g * SROW + RUN,
                    ap=[[RUN, CROP - 1], [SROW, 4], [1, RUN]],
                )
                nc.gpsimd.dma_start(
                    out=dst, in_=t[1:CROP, 4 * g * RUN : 4 * (g + 1) * RUN]
                )

            for s in range(BPC):
                col = slice(s * RUN, (s + 1) * RUN)
                # rows 0-63 as half-row pairs -> 128 descriptors (uniform
                # engine spread), on the HWDGE rings
                rk = "sync" if s % 2 == 0 else "scalar"
                eng_, reg_ = engs[rk], regs[rk]
                eng_.reg_load(reg_, offs_sb[0:1, s : s + 1])
                ov = eng_.snap(reg_, donate=True, min_val=0, max_val=MAXEOFF)
                src_a = bass.AP(
                    tensor=images[:].tensor,
                    offset=ov,
                    ap=[[USTRIDE, RSPLIT], [HALF, 2], [1, HALF]],
                    dep_tracking_offset=s * IMSZ,
                )
                eng_.dma_start(out=t[0:RSPLIT, col], in_=src_a)
                # rows 64-87 via SWDGE (uniform by construction)
                nc.gpsimd.reg_load(g_reg, offs_sb[0:1, s : s + 1])
                ovp = nc.gpsimd.snap(g_reg, donate=True, min_val=0,
                                     max_val=MAXEOFF)
                src_b = bass.AP(
                    tensor=images[:].tensor,
                    offset=ovp + RSPLIT * USTRIDE,
                    ap=[[USTRIDE, CROP - RSPLIT], [1, RUN]],
                    dep_tracking_offset=s * IMSZ + RSPLIT * USTRIDE,
                )
                nc.gpsimd.dma_start(out=t[RSPLIT:CROP, col], in_=src_b)
                # k=0 column scale (first 16 elems of the sample's run)
                nc.vector.tensor_scalar_mul(
                    t[:, s * RUN : s * RUN + C],
                    t[:, s * RUN : s * RUN + C],
                    0.5,
                )
                # emit the previous group's regroup+store once the next
                # group's reads are queued, so its sem waits are already
                # satisfied and never stall the Pool queue
                if s % 4 == 3 and s > 3:
                    pool_group_ops(s // 4 - 1)
                if s == 14:
                    # ex0/ex1 landed long ago; scale edge tile for groups 0+1
                    nc.vector.tensor_scalar_mul(tmps[0][:], tmps[0][:], 0.5)
            pool_group_ops(3)
            nc.vector.tensor_scalar_mul(tmps[1][:], tmps[1][:], 0.5)
            for g, seng in ((0, nc.sync), (1, nc.scalar)):
                dst0 = bass.AP(
                    tensor=out[:].tensor,
                    offset=g * 8 * SROW,
                    ap=[[SROW, 8], [1, RUN]],
                )
                seng.dma_start(out=dst0, in_=tmps[g][:])
    nc.finalize()
    return nc


def _get_nc():
    if "nc" not in _nc_cache:
        _nc_cache["nc"] = _build_nc()
    return _nc_cache["nc"]


def _host_offsets(locs):
    locs = np.asarray(locs, dtype=np.float32)
    t = np.clip(locs * np.float32(SCALE), np.float32(TL), np.float32(IN - TL))
    return np.floor(t - np.float32(TL)).astype(np.int32)  # [B, 2] (w, h)


def make_in_maps(images, locs):
    images = np.asarray(images, dtype=np.float32)
    off = _host_offsets(locs)  # [B, 2] (w, h)
    s_idx = np.arange(BPC, dtype=np.int64)
    in_maps = []
    for i in range(NCORES):
        sl = slice(i * BPC, (i + 1) * BPC)
        osh = off[sl].astype(np.int64)
        eoff = (s_idx * IMSZ + C * (osh[:, 0] * IN + osh[:, 1])).astype(np.int32)
        # channel-last slab [s][u][v][c]
        slab = np.ascontiguousarray(
            np.moveaxis(images[sl], 1, -1)
        ).reshape(1, -1)
        in_maps.append(
            {
                "images": slab,
                "offs": np.ascontiguousarray(eoff.reshape(1, -1)),
            }
        )
    return in_maps


def run(images, locs, trace=False, **kwargs):
    nc = _get_nc()
    in_maps = make_in_maps(images, locs)
    res = run_bass_kernel_spmd(
        nc, in_maps, core_ids=list(range(NCORES)), trace=trace, **kwargs
    )
    outs = []
    for i in range(NCORES):
        o = np.asarray(res.results[i]["out"])  # [BPC, 88, 88, C]
        outs.append(np.moveaxis(o, -1, 1))     # -> [BPC, C, 88, 88]
    full = np.ascontiguousarray(np.concatenate(outs, axis=0)).astype(np.float32)
    return full, res


def kernel(images, locs):
    full, _ = run(images, locs, trace=False)
    return full


# revision 23
# speedup vs baseline: 2.0406x; 2.0406x over previous
"""AttentionCropLayer Trainium2 kernel.

Per sample b: offsets (w,h) = floor(clip(locs[b]*224, 44, 180) - 44); output
out[b] = images[b, :, w:w+88, h:h+88] * mask, with mask the fixed 88x88
sigmoid-profile outer product.

The sigmoid profile sig(10*r) - sig(10*(r-88)) is 0.5 at r=0 and within
4.6e-5 of 1.0 for r=1..87, so the mask reduces to scaling row 0 and column 0
of each crop by 0.5 (corner 0.25); the interior passes through. Max relative
error ~1e-4, far inside the 2e-2 gate.

Data movement (HW-calibrated to run at the HBM limit with all 16 SDMA
engines uniformly busy):
  - host reorders each core's slab to channel-last [s][u][v][c] and rolls
    each sample by (h mod 4) columns so every crop row starts on a 256B
    boundary; one crop row = 16 channels = 1408 contiguous elements (5632B)
  - the output is viewed as 1408 global rows (s*88 + i) of 1408 elements,
    processed as 11 chunks of 128 rows; reads are 11 SWDGE dma_gathers
    (128 int16 indices each, elem 5632B, elem_step 256B) whose outputs land
    [partition = row-in-chunk, free = chunk column]; stores are 11 HWDGE
    128-descriptor one-per-partition DMAs - both shapes measure perfectly
    uniform across engines at full rate
  - k=0 scale: one [128,16] DVE op per chunk; i=0 rows are permuted within
    their chunk (via the gather index order; host un-permutes) onto
    partitions 0/64 where single-partition scale ops are architecturally
    legal (compute ops must start on a partition quadrant)
"""

import sys

if "/opt/trn_rl_repo" not in sys.path:
    sys.path.insert(0, "/opt/trn_rl_repo")

import numpy as np

import concourse.bass as bass
import concourse.bacc as bacc
import concourse.mybir as mybir
from concourse import tile
from concourse.bass_utils import run_bass_kernel_spmd

TL = 44
CROP = 2 * TL          # 88
SCALE = 224.0
B, C, IN = 128, 16, 224
NCORES = 8
BPC = B // NCORES      # 16 samples per core
MAXOFF = IN - CROP     # 136
IMSZ = C * IN * IN     # elements per sample
RUN = CROP * C         # 1408 elements: one crop row x all channels
USTRIDE = IN * C       # 3584
GROWS = BPC * CROP     # 1408 global output rows per core
NCHUNK = GROWS // 128  # 11
ESTEP = 64             # gather elem_step: 64 elems = 256B
# chunk j covers global rows [128j, 128j+128); the gather window starts at
# the first row's static (sample, i) base so relative indices stay in int16
# even for chunks spanning three samples
S_FIRST = [(128 * j) // CROP for j in range(NCHUNK)]
I_FIRST = [128 * j - CROP * S_FIRST[j] for j in range(NCHUNK)]
_MAXB64 = (MAXOFF * IN + MAXOFF) // 4  # 7650: largest per-sample crop base
NIDX = [
    ((128 * j + 127) // CROP - S_FIRST[j]) * (IMSZ // 64)
    + _MAXB64
    + ((128 * j + 127) % CROP) * 56
    - I_FIRST[j] * 56
    + 1
    for j in range(NCHUNK)
]

_nc_cache = {}


def _perm():
    """Within-chunk row permutation putting each i=0 row at partition 0 or
    64 of its chunk. Returns perm (dev position -> global row)."""
    perm = np.arange(GROWS, dtype=np.int64)
    for j in range(NCHUNK):
        targets = [0, 64]
        for s in range(BPC):
            g = CROP * s
            if 128 * j <= g < 128 * (j + 1):
                t = 128 * j + targets.pop(0)
                a = int(np.where(perm == g)[0][0])
                perm[a], perm[t] = perm[t], perm[a]
    return perm


PERM = _perm()
# which targets are used per chunk (for the device scale ops)
ROW0_AT = [
    [t for t in (0, 64)
     if PERM[128 * j + t] % CROP == 0]
    for j in range(NCHUNK)
]


def _build_nc():
    nc = bacc.Bacc(None)
    images = nc.declare_dram_parameter(
        "images", [1, BPC * IMSZ], mybir.dt.float32, isOutput=False
    )
    idxs = nc.declare_dram_parameter(
        "idxs", [128, 8 * NCHUNK], mybir.dt.int16, isOutput=False
    )
    out = nc.declare_dram_parameter(
        "out", [GROWS, RUN], mybir.dt.float32, isOutput=True
    )

    with tile.TileContext(nc) as tc:
        with tc.tile_pool(name="work", bufs=1) as wpool:
            T = wpool.tile([128, NCHUNK * RUN], mybir.dt.float32, tag="T")
            ix = wpool.tile([128, 8 * NCHUNK], mybir.dt.int16, tag="ix")
            nc.sync.dma_start(out=ix[:], in_=idxs[:])

            def emit_store(j):
                dst = bass.AP(
                    tensor=out[:].tensor,
                    offset=j * 128 * RUN,
                    ap=[[RUN, 128], [1, RUN]],
                )
                seng = nc.sync if j % 2 == 0 else nc.scalar
                seng.dma_start(out=dst, in_=T[:, j * RUN : (j + 1) * RUN])

            for j in range(NCHUNK):
                woff = S_FIRST[j] * IMSZ + I_FIRST[j] * USTRIDE
                src = bass.AP(
                    tensor=images[:].tensor,
                    offset=woff,
                    ap=[[ESTEP, NIDX[j]], [1, RUN]],
                    dep_tracking_offset=woff,
                )
                tap = T[:, j * RUN : (j + 1) * RUN]
                dst = bass.AP(
                    tensor=tap.tensor,
                    offset=tap.offset,
                    ap=[tap.ap[0], [RUN, 1], [1, RUN]],
                )
                nc.gpsimd.dma_gather(
                    out_ap=dst,
                    in_ap=src,
                    idxs_ap=ix[:, 8 * j : 8 * (j + 1)],
                    num_idxs=128,
                    num_idxs_reg=128,
                    elem_size=RUN,
                    elem_step=ESTEP,
                    single_packet=False,
                )
                # k=0 columns: first 16 elements of every row
                nc.vector.tensor_scalar_mul(
                    T[:, j * RUN : j * RUN + C],
                    T[:, j * RUN : j * RUN + C],
                    0.5,
                )
                # i=0 rows, permuted onto quadrant partitions 0 / 64
                for t in ROW0_AT[j]:
                    view = T[t : t + 1, j * RUN : (j + 1) * RUN]
                    if t == 0:
                        nc.vector.tensor_scalar_mul(view, view, 0.5)
                    else:
                        nc.scalar.mul(view, view, 0.5)
                # lag stores by one chunk so their sem waits are satisfied
                # by the time they reach the ring queue head
                if j > 0:
                    emit_store(j - 1)
            emit_store(NCHUNK - 1)
    nc.finalize()
    return nc


def _get_nc():
    if "nc" not in _nc_cache:
        _nc_cache["nc"] = _build_nc()
    return _nc_cache["nc"]


def _host_offsets(locs):
    locs = np.asarray(locs, dtype=np.float32)
    t = np.clip(locs * np.float32(SCALE), np.float32(TL), np.float32(IN - TL))
    return np.floor(t - np.float32(TL)).astype(np.int32)  # [B, 2] (w, h)


def make_in_maps(images, locs):
    images = np.asarray(images, dtype=np.float32)
    off = _host_offsets(locs)  # [B, 2] (w, h)
    in_maps = []
    for core in range(NCORES):
        sl = slice(core * BPC, (core + 1) * BPC)
        osh = off[sl].astype(np.int64)
        w, h = osh[:, 0], osh[:, 1]
        sh = h % 4
        hq = h - sh
        # channel-last slab, each sample rolled by (h%4) columns so crop
        # rows start on 256B boundaries
        cl = np.moveaxis(images[sl], 1, -1)  # [s, u, v, c]
        slab = np.empty_like(cl)
        for s in range(BPC):
            slab[s] = np.roll(cl[s], -int(sh[s]), axis=1)
        # gather indices, int16, in units of 64 elements (256B); idx i of
        # chunk j lives at [i % 16, 8j + i // 16], replicated across the 8
        # sixteen-partition Q7-core blocks
        base64 = (w * IN + hq) // 4          # per-sample crop base
        ix16 = np.zeros((16, 8 * NCHUNK), np.int16)
        for j in range(NCHUNK):
            for p in range(128):
                g = int(PERM[128 * j + p])
                s, i = g // CROP, g % CROP
                val = ((s - S_FIRST[j]) * (IMSZ // 64) + base64[s]
                       + (i - I_FIRST[j]) * 56)
                ix16[p % 16, 8 * j + p // 16] = val
        ix = np.tile(ix16, (8, 1))
        in_maps.append(
            {
                "images": np.ascontiguousarray(slab).reshape(1, -1),
                "idxs": ix,
            }
        )
    return in_maps


def run(images, locs, trace=False, **kwargs):
    nc = _get_nc()
    in_maps = make_in_maps(images, locs)
    res = run_bass_kernel_spmd(
        nc, in_maps, core_ids=list(range(NCORES)), trace=trace, **kwargs
    )
    outs = []
    for i in range(NCORES):
        dev = np.asarray(res.results[i]["out"])      # [1408, 1408] permuted
        unperm = np.empty_like(dev)
        unperm[PERM] = dev                           # dev row q holds PERM[q]
        o = unperm.reshape(BPC, CROP, CROP, C)       # [s, i, k, c]
        outs.append(np.moveaxis(o, -1, 1))           # -> [s, c, i, k]
    full = np.ascontiguousarray(np.concatenate(outs, axis=0)).astype(np.float32)
    return full, res


def kernel(images, locs):
    full, _ = run(images, locs, trace=False)
    return full


# revision 25
# speedup vs baseline: 2.0670x; 1.0129x over previous
"""AttentionCropLayer Trainium2 kernel.

Per sample b: offsets (w,h) = floor(clip(locs[b]*224, 44, 180) - 44); output
out[b] = images[b, :, w:w+88, h:h+88] * mask, with mask the fixed 88x88
sigmoid-profile outer product.

The sigmoid profile sig(10*r) - sig(10*(r-88)) is 0.5 at r=0 and within
4.6e-5 of 1.0 for r=1..87, so the mask reduces to scaling row 0 and column 0
of each crop by 0.5 (corner 0.25); the interior passes through. Max relative
error ~1e-4, far inside the 2e-2 gate.

Data movement (HW-calibrated to run at the HBM limit with all 16 SDMA
engines uniformly busy):
  - host reorders each core's slab to channel-last [s][u][v][c] and rolls
    each sample by (h mod 4) columns so every crop row starts on a 256B
    boundary; one crop row = 16 channels = 1408 contiguous elements (5632B)
  - the output is viewed as 1408 global rows (s*88 + i) of 1408 elements,
    processed as 11 chunks of 128 rows; reads are 11 SWDGE dma_gathers
    (128 int16 indices each, elem 5632B, elem_step 256B) whose outputs land
    [partition = row-in-chunk, free = chunk column]; stores are 11 HWDGE
    128-descriptor one-per-partition DMAs - both shapes measure perfectly
    uniform across engines at full rate
  - k=0 scale: one [128,16] DVE op per chunk; i=0 rows are permuted within
    their chunk (via the gather index order; host un-permutes) onto
    partitions 0/64 where single-partition scale ops are architecturally
    legal (compute ops must start on a partition quadrant)
"""

import sys

if "/opt/trn_rl_repo" not in sys.path:
    sys.path.insert(0, "/opt/trn_rl_repo")

import numpy as np

import concourse.bass as bass
import concourse.bacc as bacc
import concourse.mybir as mybir
from concourse import tile
from concourse.bass_utils import run_bass_kernel_spmd

TL = 44
CROP = 2 * TL          # 88
SCALE = 224.0
B, C, IN = 128, 16, 224
NCORES = 8
BPC = B // NCORES      # 16 samples per core
MAXOFF = IN - CROP     # 136
IMSZ = C * IN * IN     # elements per sample
RUN = CROP * C         # 1408 elements: one crop row x all channels
USTRIDE = IN * C       # 3584
GROWS = BPC * CROP     # 1408 global output rows per core
NCHUNK = GROWS // 128  # 11
ESTEP = 64             # gather elem_step: 64 elems = 256B
# chunk j covers global rows [128j, 128j+128); the gather window starts at
# the first row's static (sample, i) base so relative indices stay in int16
# even for chunks spanning three samples
S_FIRST = [(128 * j) // CROP for j in range(NCHUNK)]
I_FIRST = [128 * j - CROP * S_FIRST[j] for j in range(NCHUNK)]
_MAXB64 = (MAXOFF * IN + MAXOFF) // 4  # 7650: largest per-sample crop base
NIDX = [
    ((128 * j + 127) // CROP - S_FIRST[j]) * (IMSZ // 64)
    + _MAXB64
    + ((128 * j + 127) % CROP) * 56
    - I_FIRST[j] * 56
    + 1
    for j in range(NCHUNK)
]

_nc_cache = {}


def _perm():
    """Within-chunk row permutation putting each i=0 row at partition 0 or
    64 of its chunk. Returns perm (dev position -> global row)."""
    perm = np.arange(GROWS, dtype=np.int64)
    for j in range(NCHUNK):
        targets = [0, 64]
        for s in range(BPC):
            g = CROP * s
            if 128 * j <= g < 128 * (j + 1):
                t = 128 * j + targets.pop(0)
                a = int(np.where(perm == g)[0][0])
                perm[a], perm[t] = perm[t], perm[a]
    return perm


PERM = _perm()
# which targets are used per chunk (for the device scale ops)
ROW0_AT = [
    [t for t in (0, 64)
     if PERM[128 * j + t] % CROP == 0]
    for j in range(NCHUNK)
]


def _build_nc():
    nc = bacc.Bacc(None)
    images = nc.declare_dram_parameter(
        "images", [1, BPC * IMSZ], mybir.dt.float32, isOutput=False
    )
    idxs = nc.declare_dram_parameter(
        "idxs", [128, 8 * NCHUNK], mybir.dt.int16, isOutput=False
    )
    out = nc.declare_dram_parameter(
        "out", [GROWS, RUN], mybir.dt.float32, isOutput=True
    )

    with tile.TileContext(nc) as tc:
        with tc.tile_pool(name="work", bufs=1) as wpool:
            T = wpool.tile([128, NCHUNK * RUN], mybir.dt.float32, tag="T")
            ix = wpool.tile([128, 8 * NCHUNK], mybir.dt.int16, tag="ix")
            # warm-up gather against a zeroed index tile: the first SWDGE
            # gather pays a one-time ucode-load cost; absorb it while the
            # real index table is still staging
            wix = wpool.tile([128, 8], mybir.dt.int16, tag="wix")
            wt = wpool.tile([128, 64], mybir.dt.float32, tag="wt")
            nc.gpsimd.memset(wix[:], 0)
            wsrc = bass.AP(
                tensor=images[:].tensor,
                offset=0,
                ap=[[ESTEP, 2], [1, 64]],
                dep_tracking_offset=0,
            )
            wtap = wt[:]
            wdst = bass.AP(
                tensor=wtap.tensor,
                offset=wtap.offset,
                ap=[wtap.ap[0], [64, 1], [1, 64]],
            )
            nc.gpsimd.dma_gather(
                out_ap=wdst,
                in_ap=wsrc,
                idxs_ap=wix[:],
                num_idxs=128,
                num_idxs_reg=128,
                elem_size=64,
                elem_step=ESTEP,
                single_packet=False,
            )
            nc.sync.dma_start(out=ix[:], in_=idxs[:])

            def emit_store(j):
                dst = bass.AP(
                    tensor=out[:].tensor,
                    offset=j * 128 * RUN,
                    ap=[[RUN, 128], [1, RUN]],
                )
                seng = nc.sync if j % 2 == 0 else nc.scalar
                seng.dma_start(out=dst, in_=T[:, j * RUN : (j + 1) * RUN])

            for j in range(NCHUNK):
                woff = S_FIRST[j] * IMSZ + I_FIRST[j] * USTRIDE
                src = bass.AP(
                    tensor=images[:].tensor,
                    offset=woff,
                    ap=[[ESTEP, NIDX[j]], [1, RUN]],
                    dep_tracking_offset=woff,
                )
                tap = T[:, j * RUN : (j + 1) * RUN]
                dst = bass.AP(
                    tensor=tap.tensor,
                    offset=tap.offset,
                    ap=[tap.ap[0], [RUN, 1], [1, RUN]],
                )
                nc.gpsimd.dma_gather(
                    out_ap=dst,
                    in_ap=src,
                    idxs_ap=ix[:, 8 * j : 8 * (j + 1)],
                    num_idxs=128,
                    num_idxs_reg=128,
                    elem_size=RUN,
                    elem_step=ESTEP,
                    single_packet=False,
                )
                # lag stores by one chunk so their sem waits are satisfied
                # by the time they reach the ring queue head
                if j > 0:
                    emit_store(j - 1)
                # k=0 columns (first 16 elements of every row) on DVE; the
                # i=0 row bulk on the otherwise-idle Act engine in parallel,
                # with its first 16 elements (the corner) as a separate tiny
                # DVE op so the two engines never touch the same region
                nc.vector.tensor_scalar_mul(
                    T[:, j * RUN : j * RUN + C],
                    T[:, j * RUN : j * RUN + C],
                    0.5,
                )
                for t in ROW0_AT[j]:
                    rest = T[t : t + 1, j * RUN + C : (j + 1) * RUN]
                    nc.scalar.mul(rest, rest, 0.5)
                    corner = T[t : t + 1, j * RUN : j * RUN + C]
                    nc.vector.tensor_scalar_mul(corner, corner, 0.5)
            emit_store(NCHUNK - 1)
    nc.finalize()
    return nc


def _get_nc():
    if "nc" not in _nc_cache:
        _nc_cache["nc"] = _build_nc()
    return _nc_cache["nc"]


def _host_offsets(locs):
    locs = np.asarray(locs, dtype=np.float32)
    t = np.clip(locs * np.float32(SCALE), np.float32(TL), np.float32(IN - TL))
    return np.floor(t - np.float32(TL)).astype(np.int32)  # [B, 2] (w, h)


def make_in_maps(images, locs):
    images = np.asarray(images, dtype=np.float32)
    off = _host_offsets(locs)  # [B, 2] (w, h)
    in_maps = []
    for core in range(NCORES):
        sl = slice(core * BPC, (core + 1) * BPC)
        osh = off[sl].astype(np.int64)
        w, h = osh[:, 0], osh[:, 1]
        sh = h % 4
        hq = h - sh
        # channel-last slab, each sample rolled by (h%4) columns so crop
        # rows start on 256B boundaries
        cl = np.moveaxis(images[sl], 1, -1)  # [s, u, v, c]
        slab = np.empty_like(cl)
        for s in range(BPC):
            slab[s] = np.roll(cl[s], -int(sh[s]), axis=1)
        # gather indices, int16, in units of 64 elements (256B); idx i of
        # chunk j lives at [i % 16, 8j + i // 16], replicated across the 8
        # sixteen-partition Q7-core blocks
        base64 = (w * IN + hq) // 4          # per-sample crop base
        ix16 = np.zeros((16, 8 * NCHUNK), np.int16)
        for j in range(NCHUNK):
            for p in range(128):
                g = int(PERM[128 * j + p])
                s, i = g // CROP, g % CROP
                val = ((s - S_FIRST[j]) * (IMSZ // 64) + base64[s]
                       + (i - I_FIRST[j]) * 56)
                ix16[p % 16, 8 * j + p // 16] = val
        ix = np.tile(ix16, (8, 1))
        in_maps.append(
            {
                "images": np.ascontiguousarray(slab).reshape(1, -1),
                "idxs": ix,
            }
        )
    return in_maps


def run(images, locs, trace=False, **kwargs):
    nc = _get_nc()
    in_maps = make_in_maps(images, locs)
    res = run_bass_kernel_spmd(
        nc, in_maps, core_ids=list(range(NCORES)), trace=trace, **kwargs
    )
    outs = []
    for i in range(NCORES):
        dev = np.asarray(res.results[i]["out"])      # [1408, 1408] permuted
        unperm = np.empty_like(dev)
        unperm[PERM] = dev                           # dev row q holds PERM[q]
        o = unperm.reshape(BPC, CROP, CROP, C)       # [s, i, k, c]
        outs.append(np.moveaxis(o, -1, 1))           # -> [s, c, i, k]
    full = np.ascontiguousarray(np.concatenate(outs, axis=0)).astype(np.float32)
    return full, res


def kernel(images, locs):
    full, _ = run(images, locs, trace=False)
    return full
